# revision 34
# baseline (speedup 1.0000x reference)
"""Trainium2 Bass kernel for nn_Attention (dense transformer attention layer).

Sharding: 8 cores = 2 (batch) x 4 (head-group TP).  Core c handles batch
c//4 and heads [4*(c%4), 4*(c%4)+4).  Each core computes LayerNorm (in the
transposed domain, folded into augmented projection matmuls), q/k/v
projections, per-head RMS-norm'd attention, and a partial output
projection; the host sums the 4 partials per batch.

Precision: the dots have sigma~512 (scale=sqrt(dh) applied to BOTH q and k),
so the q/k path needs ~fp32 accuracy.  The host splits x and Wq/Wk into
bf16 hi/lo pairs; projections run as 3 bf16 passes (xh*wh + xl*wh + xh*wl,
3 cyc/col vs fp32's 4); QK^T runs as the exact bf16x2 pack scheme
(q=[hi;lo], k1=[hi;hi], k2=[lo;lo] -> all 4 cross terms in 2 bf16 matmuls).
Per-token/column scale factors (rk, rq, 1/sum) stay fp32 because scale
errors are amplified by |dots|~1700.  The v path, attention weights, and
Wo run in bf16.

Layout notes:
 - x is fed transposed (x^T [DIM, S]) so all matmuls contract over
   partitions without on-device transposition of x.
 - LayerNorm: token sums via bf16 ones-matmuls over xh+xl; sums of squares
   via a bf16 ones-matmul over square(xh); folded into the projections
   via two augmented contraction rows: z = [x^T; colsums; 1/r] with the
   aug matmul in f32r, W = [ln_w*W; -u/1024; ln_b@W].  The per-token 1/r
   cancels in q/k (RMSNorm scale invariance) and is applied explicitly to v.
 - v is projected in NATURAL orientation (stationary = bf16 x^T chunk,
   moving = Wv) so no PE transposes of v are needed; each v_nat tile also
   carries a ones column per head so the attention AV matmul produces the
   softmax denominators for free in psum row 64.
 - The bf16x2 packs are built with SBUF->SBUF DMAs (partition moves), no
   DRAM roundtrip.
 - RMSNorm of q is folded into the softmax exp (ACT scale/bias are
   per-partition APs); RMSNorm of k and gamma_k are applied via a single
   fused scalar_tensor_tensor pass.
"""
import numpy as np
import ml_dtypes
from contextlib import ExitStack

import concourse.bass as bass
import concourse.tile as tile
from concourse import mybir
from concourse.bass_utils import run_bass_kernel_spmd
from concourse.masks import make_identity

F32 = mybir.dt.float32
F32R = mybir.dt.float32r
BF16 = mybir.dt.bfloat16
AF = mybir.ActivationFunctionType
ALU = mybir.AluOpType
AX = mybir.AxisListType

B, S, DIM, H, DH = 2, 2048, 1024, 16, 64
NCORES = 8
HPC = 4                  # heads per core
INC = HPC * DH           # 256 inner dims per core
KCH = DIM // 128         # 8 contraction chunks of x
NT = 2                   # q/k^T tiles per core ([128, S] each, 2 heads per tile)
SCH = S // 128           # 16 token chunks
NJ = S // 512            # 4 moving chunks
VW = DH + 1              # v_nat column stride per head (64 v dims + ones col)

_TPB_ENGINES = None


def _fix_multiwaits(nc, max_waits=1):
    """walrus in this container encodes at most one semaphore wait per TPB
    instruction; split extras onto single-wait NoOps ahead of the
    instruction (same engine => program order preserves semantics)."""
    global _TPB_ENGINES
    if _TPB_ENGINES is None:
        _TPB_ENGINES = {
            mybir.EngineType.PE,
            mybir.EngineType.Activation,
            mybir.EngineType.DVE,
            mybir.EngineType.Pool,
            mybir.EngineType.SP,
        }
    nsplit = 0
    for f in nc.m.functions:
        for bb in f.blocks:
            new = []
            for inst in bb.instructions:
                si = inst.sync_info
                if (
                    inst.engine in _TPB_ENGINES
                    and si is not None
                    and si.on_wait
                    and len(si.on_wait) > max_waits
                ):
                    waits = list(si.on_wait)
                    extra, keep = waits[:-max_waits], waits[-max_waits:]
                    for w in extra:
                        nop = mybir.InstNoOp(
                            name=f"I-{nc.next_id()}",
                            ins=[],
                            outs=[],
                            engine=inst.engine,
                            sync_info=mybir.SyncInfo(on_wait=[w], on_update=[]),
                        )
                        try:
                            nc.register_instruction(nop, overwrite=True)
                        except Exception:
                            pass
                        new.append(nop)
                    try:
                        si.on_wait[:] = keep
                    except TypeError:
                        inst.sync_info = mybir.SyncInfo(
                            on_wait=keep, on_update=si.on_update
                        )
                    nsplit += 1
                new.append(inst)
            bb.instructions[:] = new
    return nsplit


def _build_program():
    nc = bass.Bass("TRN2", target_bir_lowering=False, debug=False,
                   num_devices=NCORES)
    din = lambda n, s, d: nc.dram_tensor(n, list(s), d, kind="ExternalInput").ap()
    xT_d = din("xT", (DIM, S), F32R)
    wqh_d = din("Wqh", (DIM, INC), BF16)
    wql_d = din("Wql", (DIM, INC), BF16)
    wkh_d = din("Wkh", (DIM, INC), BF16)
    wkl_d = din("Wkl", (DIM, INC), BF16)
    wqa_d = din("Wqa", (2, INC), F32R)
    wka_d = din("Wka", (2, INC), F32R)
    wv_d = din("Wv", (9 * 128, INC), BF16)
    wo_d = din("Wo", (INC, DIM), BF16)
    gq_d = din("gq", (INC, 1), F32)
    gk_d = din("gk", (INC, 1), F32)
    e2_d = din("E2", (128, 2), F32R)
    e2t_d = din("E2T", (2, 128), F32R)
    e2tf_d = din("E2TF", (2, 128), F32)
    out_d = nc.dram_tensor("out", [S, DIM], F32, kind="ExternalOutput").ap()
    import os
    DBG = bool(os.environ.get("KDBG"))
    if DBG:
        dbg_qh = nc.dram_tensor("dbg_qh", [128, S], F32,
                                kind="ExternalOutput").ap()
        dbg_kh = nc.dram_tensor("dbg_kh", [128, S], F32,
                                kind="ExternalOutput").ap()
        dbg_qp = nc.dram_tensor("dbg_qp", [128, S], BF16,
                                kind="ExternalOutput").ap()
        dbg_k1 = nc.dram_tensor("dbg_k1", [128, S], BF16,
                                kind="ExternalOutput").ap()
        dbg_vn = nc.dram_tensor("dbg_vn", [128, HPC * VW], BF16,
                                kind="ExternalOutput").ap()
        dbg_sm = nc.dram_tensor("dbg_sm", [1, S], F32,
                                kind="ExternalOutput").ap()
        dbg_ag = nc.dram_tensor("dbg_ag", [2, S], F32,
                                kind="ExternalOutput").ap()
        dbg_rs = nc.dram_tensor("dbg_rs", [1, S], F32,
                                kind="ExternalOutput").ap()
        dbg_xh = nc.dram_tensor("dbg_xh", [128, S], BF16,
                                kind="ExternalOutput").ap()
        dbg_wv = nc.dram_tensor("dbg_wv", [128, INC], BF16,
                                kind="ExternalOutput").ap()

    with tile.TileContext(nc) as tc, ExitStack() as ctx, \
            nc.allow_low_precision(reason="f32r/bf16 rounding intended"):
        # ---- long-lived pools (entered first: strict LIFO pool release)
        consts = ctx.enter_context(tc.tile_pool(name="consts", bufs=1))
        qkpool = ctx.enter_context(tc.tile_pool(name="qk", bufs=1))
        vpool = ctx.enter_context(tc.tile_pool(name="v", bufs=1))
        opool = ctx.enter_context(tc.tile_pool(name="o", bufs=1))

        ident = consts.tile([128, 128], BF16, tag="ident", name="ident")
        make_identity(nc, ident)
        one1 = consts.tile([1, 1], F32, tag="one1", name="one1")
        nc.vector.memset(one1, 1.0)
        eps1 = consts.tile([1, 1], F32, tag="eps1", name="eps1")
        nc.vector.memset(eps1, 1e-5)
        c15 = consts.tile([2, 1], F32, tag="c15", name="c15")
        nc.vector.memset(c15, 1.5)
        eps8 = consts.tile([2, 1], F32, tag="eps8", name="eps8")
        nc.vector.memset(eps8, 1e-8)
        ones_b = consts.tile([128, 1], BF16, tag="ones", name="ones")
        nc.vector.memset(ones_b, 1.0)
        e2 = consts.tile([128, 2], F32R, tag="e2", name="e2")
        nc.sync.dma_start(out=e2, in_=e2_d[:])
        e2f = consts.tile([128, 2], F32, tag="e2f", name="e2f")
        nc.gpsimd.dma_start(out=e2f, in_=e2)
        e2t2r = consts.tile([2, 128], F32R, tag="e2t2r", name="e2t2r")
        nc.sync.dma_start(out=e2t2r, in_=e2t_d[:])
        e2t2f = consts.tile([2, 128], F32, tag="e2t2f", name="e2t2f")
        nc.sync.dma_start(out=e2t2f, in_=e2tf_d[:])
        r_col = consts.tile([128, SCH], F32, tag="r_col", name="r_col")
        gq_t = [consts.tile([128, 1], F32, tag=f"gq{t}", name=f"gq{t}")
                for t in range(NT)]
        gk_t = [consts.tile([128, 1], F32, tag=f"gk{t}", name=f"gk{t}")
                for t in range(NT)]
        for t in range(NT):
            nc.sync.dma_start(out=gq_t[t], in_=gq_d[t * 128:(t + 1) * 128, :])
            nc.sync.dma_start(out=gk_t[t], in_=gk_d[t * 128:(t + 1) * 128, :])
        # rmsnorm factors for q (per-partition layout; col = 2*sc+hh)
        rq_all = [consts.tile([128, 2 * SCH], F32, tag=f"rq{t}", name=f"rq{t}")
                  for t in range(NT)]
        nrq_all = [consts.tile([128, 2 * SCH], F32, tag=f"nrq{t}", name=f"nrq{t}")
                   for t in range(NT)]

        qhat = [qkpool.tile([128, S], F32, tag=f"qh{t}", name=f"qh{t}")
                for t in range(NT)]
        khat = [qkpool.tile([128, S], F32, tag=f"kh{t}", name=f"kh{t}")
                for t in range(NT)]
        # v in natural orientation [tokens, 4 heads x (64 dims + ones col)]
        v_nat = [vpool.tile([128, HPC * VW], BF16, tag=f"vn{j}", name=f"vn{j}")
                 for j in range(SCH)]
        outT = [opool.tile([128, S], BF16, tag=f"oT{k}", name=f"oT{k}")
                for k in range(NT)]
        # softmax denominators, one row tile per head, filled by phase F
        sums4 = [opool.tile([1, S], F32R, tag=f"sums{h}", name=f"sums{h}")
                 for h in range(HPC)]

        with ExitStack() as phase_pre:
            augpool = phase_pre.enter_context(tc.tile_pool(name="aug", bufs=1))
            aug_f = augpool.tile([2, S], F32R, tag="aug_f", name="aug_f")
            r_sb = augpool.tile([1, S], F32, tag="r_sb", name="r_sb")
            wvpool = phase_pre.enter_context(tc.tile_pool(name="wv", bufs=1))
            wv = [wvpool.tile([128, INC], BF16, tag=f"wv{k}", name=f"wv{k}")
                  for k in range(9)]
            aug_b = wvpool.tile([2, S], BF16, tag="aug_b", name="aug_b")

            with ExitStack() as phase_bx:
                xpool = phase_bx.enter_context(tc.tile_pool(name="x", bufs=1))
                xh = [xpool.tile([128, S], BF16, tag=f"xh{k}", name=f"xh{k}")
                      for k in range(KCH)]
                xl = [xpool.tile([128, S], BF16, tag=f"xl{k}", name=f"xl{k}")
                      for k in range(KCH)]

                # ---- phase B: LayerNorm stats -----------------------------
                with ExitStack() as phase_b:
                    x2pool = phase_b.enter_context(
                        tc.tile_pool(name="x2", bufs=1))
                    browp = phase_b.enter_context(
                        tc.tile_pool(name="brow", bufs=1))
                    stps = phase_b.enter_context(
                        tc.tile_pool(name="stps", bufs=1, space="PSUM"))
                    sums_ps = stps.tile([1, S], F32, tag="sums", name="sums")
                    sumsq_ps = stps.tile([1, S], F32, tag="sumsq", name="sumsq")
                    xtp = phase_b.enter_context(
                        tc.tile_pool(name="xtp", bufs=2))
                    ones_f = consts.tile([128, 1], F32R, tag="ones_f",
                                         name="ones_f")
                    nc.vector.tensor_add(ones_f, e2[:, 0:1], e2[:, 1:2])
                    for k in range(KCH):
                        xt = xtp.tile([128, S], F32R, tag="xt", name="xt")
                        nc.sync.dma_start(out=xt,
                                          in_=xT_d[k * 128:(k + 1) * 128, :])
                        # on-device bf16 hi/lo split (large bf16 inputs get
                        # mangled by the XLA 16-bit tiled layout)
                        nc.scalar.copy(xh[k], xt)
                        nc.vector.tensor_sub(xl[k], xt, xh[k])
                        x2 = x2pool.tile([128, S], BF16, tag="x2", name="x2")
                        nc.scalar.square(x2, xh[k])
                        for n in range(NJ):
                            nsl = slice(n * 512, (n + 1) * 512)
                            nc.tensor.matmul(sums_ps[:, nsl], ones_f,
                                             xt[:, nsl],
                                             start=(k == 0),
                                             stop=(k == KCH - 1),
                                             skip_group_check=True)
                        for n in range(NJ):
                            nsl = slice(n * 512, (n + 1) * 512)
                            nc.tensor.matmul(sumsq_ps[:, nsl], ones_b,
                                             x2[:, nsl],
                                             start=(k == 0),
                                             stop=(k == KCH - 1),
                                             skip_group_check=True)
                    for k in range(9):
                        nc.sync.dma_start(out=wv[k],
                                          in_=wv_d[k * 128:(k + 1) * 128, :])
                    # sigma = sqrt((sumsq - sums^2/1024)/1024 + 1e-5).
                    # [1,S] rows run on ONE lane, so minimize row ops and
                    # fold scales into ACT square/sqrt.
                    nc.scalar.activation(out=r_sb, in_=sums_ps,
                                         func=AF.Square, bias=0.0,
                                         scale=1.0 / 32.0)
                    nc.vector.tensor_sub(r_sb, sumsq_ps, r_sb)
                    sig_row = browp.tile([1, S], F32, tag="sig", name="sig")
                    nc.scalar.activation(out=sig_row, in_=r_sb, func=AF.Sqrt,
                                         bias=eps1[0:1, 0:1],
                                         scale=1.0 / DIM)
                    nc.vector.tensor_copy(aug_f[0:1, :], sums_ps)
                    # invr = sigma  (aug row 1; gpsimd dma does the cast)
                    nc.gpsimd.dma_start(out=aug_f[1:2, :], in_=sig_row)
                nc.gpsimd.dma_start(out=aug_b, in_=aug_f)
                if DBG:
                    nc.gpsimd.dma_start(out=dbg_ag, in_=aug_f)
                    nc.sync.dma_start(out=dbg_rs, in_=r_sb)
                    nc.sync.dma_start(out=dbg_xh, in_=xh[0])
                    nc.sync.dma_start(out=dbg_wv, in_=wv[0])

                # r as a column [128 tokens, SCH]
                with ExitStack() as phase_rc:
                    rtp = phase_rc.enter_context(
                        tc.tile_pool(name="rtp", bufs=2, space="PSUM"))
                    sig_col = consts.tile([128, SCH], F32, tag="sig_col",
                                          name="sig_col")
                    for j in range(SCH):
                        rp = rtp.tile([128, 1], F32, tag="rp", name="rp")
                        nc.tensor.transpose(rp,
                                            sig_row[0:1,
                                                    j * 128:(j + 1) * 128],
                                            one1[0:1, 0:1])
                        nc.vector.tensor_copy(sig_col[:, j:j + 1], rp)
                    nc.vector.reciprocal(r_col, sig_col)

                # ---- phase C: q/k projections (3-pass bf16x2) -------------
                with ExitStack() as phase_c:
                    wqkpool = phase_c.enter_context(
                        tc.tile_pool(name="wqk", bufs=1))
                    prps = phase_c.enter_context(
                        tc.tile_pool(name="prps", bufs=4, space="PSUM"))
                    for wh_d, wl_d, wa_d, dst in (
                            (wqh_d, wql_d, wqa_d, qhat),
                            (wkh_d, wkl_d, wka_d, khat)):
                        wh = [wqkpool.tile([128, INC], BF16, tag=f"wh{k}",
                                           name=f"wh{k}") for k in range(KCH)]
                        wl = [wqkpool.tile([128, INC], BF16, tag=f"wl{k}",
                                           name=f"wl{k}") for k in range(KCH)]
                        wa = wqkpool.tile([2, INC], F32R, tag="wa", name="wa")
                        for k in range(KCH):
                            nc.sync.dma_start(
                                out=wh[k], in_=wh_d[k * 128:(k + 1) * 128, :])
                            nc.sync.dma_start(
                                out=wl[k], in_=wl_d[k * 128:(k + 1) * 128, :])
                        nc.sync.dma_start(out=wa, in_=wa_d[:])
                        for m in range(NT):
                            msl = slice(m * 128, (m + 1) * 128)
                            for n in range(NJ):
                                nsl = slice(n * 512, (n + 1) * 512)
                                ps = prps.tile([128, 512], F32, tag="proj",
                                               name="proj")
                                for k in range(KCH):
                                    nc.tensor.matmul(ps, wh[k][:, msl],
                                                     xh[k][:, nsl],
                                                     start=(k == 0),
                                                     stop=False)
                                    nc.tensor.matmul(ps, wh[k][:, msl],
                                                     xl[k][:, nsl],
                                                     start=False, stop=False)
                                    nc.tensor.matmul(ps, wl[k][:, msl],
                                                     xh[k][:, nsl],
                                                     start=False, stop=False)
                                nc.tensor.matmul(ps, wa[0:2, msl],
                                                 aug_f[:, nsl],
                                                 start=False, stop=True)
                                if n % 2 == 0:
                                    nc.vector.tensor_copy(dst[m][:, nsl], ps)
                                else:
                                    nc.scalar.copy(dst[m][:, nsl], ps)

                # ---- phase E: v projection (bf16, natural orientation) ----
                with ExitStack() as phase_e:
                    vprps = phase_e.enter_context(
                        tc.tile_pool(name="vprps", bufs=2, space="PSUM"))
                    for jc in range(SCH):
                        jsl = slice(jc * 128, (jc + 1) * 128)
                        vp = vprps.tile([128, INC], F32, tag="vp", name="vp")
                        for k in range(KCH):
                            nc.tensor.matmul(vp, xh[k][:, jsl], wv[k],
                                             start=(k == 0), stop=False)
                        nc.tensor.matmul(vp, aug_b[:, jsl], wv[8][0:2, :],
                                         start=False, stop=True)
                        # fold 1/sigma into the psum->sbuf copy; strided dst
                        # leaves a ones column per head for the denominator
                        dst = bass.AP(
                            tensor=v_nat[jc].tensor,
                            offset=v_nat[jc].offset,
                            ap=[v_nat[jc].ap[0], [VW, HPC], [1, DH]],
                        )
                        src = vp[:].rearrange("p (h d) -> p h d", h=HPC)
                        nc.vector.tensor_scalar_mul(dst, src,
                                                    r_col[:, jc:jc + 1])
                        onescol = bass.AP(
                            tensor=v_nat[jc].tensor,
                            offset=v_nat[jc].offset + DH,
                            ap=[v_nat[jc].ap[0], [VW, HPC], [1, 1]],
                        )
                        nc.gpsimd.memset(onescol, 1.0)

            # ---- phase D: rmsnorm factors -----------------------------
            with ExitStack() as phase_d:
                sq2pool = phase_d.enter_context(
                    tc.tile_pool(name="sq2", bufs=2))
                dstage = phase_d.enter_context(
                    tc.tile_pool(name="dstage", bufs=1))
                ssqps = phase_d.enter_context(
                    tc.tile_pool(name="ssqps", bufs=2, space="PSUM"))
                sskps = phase_d.enter_context(
                    tc.tile_pool(name="sskps", bufs=1, space="PSUM"))
                kmps = phase_d.enter_context(
                    tc.tile_pool(name="kmps", bufs=2, space="PSUM"))
                for t in range(NT):
                    q2 = sq2pool.tile([128, S], F32R, tag="q2", name="q2")
                    nc.gpsimd.tensor_mul(q2, qhat[t], qhat[t])
                    for sc in range(SCH):
                        psq = ssqps.tile([128, 2], F32, tag="ssq", name="ssq")
                        nc.tensor.matmul(psq, q2[:, sc * 128:(sc + 1) * 128],
                                         e2, start=True, stop=True)
                        nc.vector.tensor_copy(
                            rq_all[t][:, 2 * sc:2 * sc + 2], psq)
                    # rq = rsqrt(ss/64 + 1e-8), Newton-refined
                    tq = dstage.tile([128, 2 * SCH], F32, tag="tq", name="tq")
                    nc.vector.tensor_scalar(tq, rq_all[t], 1.0 / DH, 1e-8,
                                            op0=ALU.mult, op1=ALU.add)
                    nc.scalar.activation(out=rq_all[t], in_=tq, func=AF.Sqrt,
                                         bias=0.0, scale=1.0)
                    nc.vector.reciprocal(rq_all[t], rq_all[t])
                    zz = dstage.tile([128, 2 * SCH], F32, tag="zz", name="zz")
                    nc.vector.tensor_mul(zz, rq_all[t], rq_all[t])
                    nc.vector.tensor_mul(zz, zz, tq)
                    nc.vector.tensor_scalar(zz, zz, -0.5, 1.5,
                                            op0=ALU.mult, op1=ALU.add)
                    nc.vector.tensor_mul(rq_all[t], rq_all[t], zz)
                    nc.vector.tensor_scalar_mul(nrq_all[t], rq_all[t], -1.0)
                    # gamma*8 on q
                    nc.vector.tensor_scalar_mul(qhat[t], qhat[t], gq_t[t])

                    k2t = sq2pool.tile([128, S], F32, tag="q2", name="k2t")
                    nc.gpsimd.tensor_mul(k2t, khat[t], khat[t])
                    psk = sskps.tile([2, S], F32, tag="ssk", name="ssk")
                    for n in range(NJ):
                        nsl = slice(n * 512, (n + 1) * 512)
                        nc.tensor.matmul(psk[:, nsl], e2f, k2t[:, nsl],
                                         start=True, stop=True,
                                         skip_group_check=True)
                    # rk = rsqrt(psk/64 + 1e-8).  [2,S] rows run on <=2
                    # DVE lanes, so gather the rows across 32 partitions via
                    # SBUF<->SBUF DMA, do sqrt+recip+Newton there (fast),
                    # and scatter back to rows for the broadcast matmul.
                    tks = dstage.tile([2, S], F32, tag="tks", name="tks")
                    nc.vector.tensor_scalar(tks, psk, 1.0 / DH, 1e-8,
                                            op0=ALU.mult, op1=ALU.add)
                    tg = dstage.tile([32, 128], F32, tag="tg", name="tg")
                    for rr in range(2):
                        nc.sync.dma_start(
                            out=tg[16 * rr:16 * rr + 16, :],
                            in_=tks[rr:rr + 1, :].rearrange(
                                "p (a b) -> p a b", a=16))
                    rg = dstage.tile([32, 128], F32, tag="rg", name="rg")
                    nc.scalar.activation(out=rg, in_=tg, func=AF.Sqrt,
                                         bias=0.0, scale=1.0)
                    nc.vector.reciprocal(rg, rg)
                    zg = dstage.tile([32, 128], F32, tag="zg", name="zg")
                    nc.vector.tensor_mul(zg, rg, rg)
                    nc.vector.tensor_mul(zg, zg, tg)
                    nc.vector.tensor_scalar(zg, zg, -0.5, 1.5,
                                            op0=ALU.mult, op1=ALU.add)
                    nc.vector.tensor_mul(rg, rg, zg)
                    rk2 = dstage.tile([2, S], F32, tag="rk2", name="rk2")
                    for rr in range(2):
                        nc.scalar.dma_start(
                            out=rk2[rr:rr + 1, :].rearrange(
                                "p (a b) -> p a b", a=16),
                            in_=rg[16 * rr:16 * rr + 16, :])
                    # k *= gamma*8 (per-partition) and rk (fp32 broadcast over
                    # partitions via K=2 matmul), fused in one DVE pass
                    for n in range(NJ):
                        nsl = slice(n * 512, (n + 1) * 512)
                        km = kmps.tile([128, 512], F32, tag="km", name="km")
                        nc.tensor.matmul(km, e2t2f, rk2[:, nsl],
                                         start=True, stop=True)
                        nc.vector.scalar_tensor_tensor(
                            khat[t][:, nsl], khat[t][:, nsl], gk_t[t], km,
                            op0=ALU.mult, op1=ALU.mult)

        # ---- bf16x2 packs + attention --------------------------------------
        with ExitStack() as phase_post:
            packp = phase_post.enter_context(tc.tile_pool(name="packs",
                                                          bufs=1))
            hilop = phase_post.enter_context(tc.tile_pool(name="hilo",
                                                          bufs=1))

            # ---- phase F: attention (pack build per t, head chains
            # interleaved at the u level) ---------------------------------
            with ExitStack() as phase_f:
                dots_pool = phase_f.enter_context(
                    tc.tile_pool(name="dots", bufs=5, space="PSUM"))
                tpps = phase_f.enter_context(
                    tc.tile_pool(name="tpps", bufs=1, space="PSUM"))
                avps = phase_f.enter_context(
                    tc.tile_pool(name="avps", bufs=1, space="PSUM"))
                attn_pool = phase_f.enter_context(tc.tile_pool(name="attn",
                                                               bufs=2))
                attnT_pool = phase_f.enter_context(tc.tile_pool(name="attnT",
                                                                bufs=2))
                small = phase_f.enter_context(tc.tile_pool(name="small",
                                                           bufs=8))

                def flush_av(pend, part):
                    # part<4: emit av matmul chunk (4 of 16 jc) for the
                    # pending sup; part==4 -> psum->sbuf copies
                    if pend is None:
                        return
                    p_attnT, p_av, p_t, p_hh, p_sup = pend
                    p_h4 = 2 * p_t + p_hh
                    if part < 4:
                        for jc in range(4 * part, 4 * part + 4):
                            nc.tensor.matmul(
                                p_av,
                                v_nat[jc][:, p_h4 * VW:(p_h4 + 1) * VW],
                                p_attnT[:, jc * 512:(jc + 1) * 512],
                                start=(jc == 0), stop=(jc == SCH - 1))
                    else:
                        poff = p_hh * 64
                        pssl = slice(p_sup * 512, (p_sup + 1) * 512)
                        if p_sup % 2 == 0:
                            nc.vector.tensor_copy(
                                outT[p_t][poff:poff + 64, pssl],
                                p_av[0:DH, :])
                            nc.scalar.copy(sums4[p_h4][:, pssl],
                                           p_av[DH:DH + 1, :])
                        else:
                            nc.scalar.copy(
                                outT[p_t][poff:poff + 64, pssl],
                                p_av[0:DH, :])
                            nc.vector.tensor_copy(sums4[p_h4][:, pssl],
                                                  p_av[DH:DH + 1, :])

                pending = {0: None, 1: None}
                for t in range(NT):
                    # bf16x2 hi/lo split of this t's scaled q/k and
                    # partition-moving pack DMAs (SBUF->SBUF, 3 queues)
                    qhi = hilop.tile([128, S], BF16, tag="qhi", name="qhi")
                    qlo = hilop.tile([128, S], BF16, tag="qlo", name="qlo")
                    khi = hilop.tile([128, S], BF16, tag="khi", name="khi")
                    klo = hilop.tile([128, S], BF16, tag="klo", name="klo")
                    nc.scalar.copy(qhi, qhat[t])
                    nc.vector.tensor_sub(qlo, qhat[t], qhi)
                    nc.scalar.copy(khi, khat[t])
                    nc.vector.tensor_sub(klo, khat[t], khi)
                    q_pack, k1_pack, k2_pack = {}, {}, {}
                    qs = [nc.sync, nc.scalar, nc.gpsimd]
                    qi = 0
                    for hh in range(2):
                        rows = slice(hh * 64, hh * 64 + 64)
                        q_pack[hh] = packp.tile([128, S], BF16,
                                                tag=f"qp{hh}",
                                                name=f"qp{hh}")
                        k1_pack[hh] = packp.tile([128, S], BF16,
                                                 tag=f"k1p{hh}",
                                                 name=f"k1p{hh}")
                        k2_pack[hh] = packp.tile([128, S], BF16,
                                                 tag=f"k2p{hh}",
                                                 name=f"k2p{hh}")
                        moves = [
                            (q_pack[hh][0:64, :], qhi[rows, :]),
                            (q_pack[hh][64:128, :], qlo[rows, :]),
                            (k1_pack[hh][0:64, :], khi[rows, :]),
                            (k1_pack[hh][64:128, :], khi[rows, :]),
                            (k2_pack[hh][0:64, :], klo[rows, :]),
                            (k2_pack[hh][64:128, :], klo[rows, :]),
                        ]
                        for o, i in moves:
                            qs[qi % 3].dma_start(out=o, in_=i)
                            qi += 1
                    for sup in range(SCH // 4):
                        attnTs = {}
                        for hh in range(2):
                            attnTs[hh] = attnT_pool.tile(
                                [128, 4 * S], BF16, tag=f"attnT{hh}",
                                name=f"attnT{hh}")
                        for u in range(4):
                            for hh in range(2):
                                h4 = 2 * t + hh
                                attnT = attnTs[hh]
                                ic = sup * 4 + u
                                isl = slice(ic * 128, (ic + 1) * 128)
                                col = slice(2 * ic + hh, 2 * ic + hh + 1)
                                dots = [dots_pool.tile([128, 512], F32,
                                                       tag="dots",
                                                       name="dots")
                                        for _ in range(NJ)]
                                for jn in range(NJ):
                                    jsl = slice(jn * 512, (jn + 1) * 512)
                                    nc.tensor.matmul(
                                        dots[jn], q_pack[hh][:, isl],
                                        k1_pack[hh][:, jsl],
                                        start=True, stop=False,
                                        skip_group_check=True)
                                    nc.tensor.matmul(
                                        dots[jn], q_pack[hh][:, isl],
                                        k2_pack[hh][:, jsl],
                                        start=False, stop=True,
                                        skip_group_check=True)
                                # PE stall-filler: one av chunk of the
                                # previous sup between dots and transposes
                                flush_av(pending[hh], u)
                                mx = [small.tile([128, 1], F32, tag=f"mx{j}",
                                                 name=f"mx{j}")
                                      for j in range(NJ)]
                                for jn in range(NJ):
                                    nc.vector.tensor_reduce(out=mx[jn],
                                                            in_=dots[jn],
                                                            axis=AX.X,
                                                            op=ALU.max)
                                    if jn in (1, 3):
                                        # HAM keep-warm: a 1-column
                                        # LDWEIGHTS gated on a reduce pulses
                                        # the PE mid-gap so the idle window
                                        # never completes (K=8/8)
                                        mxb = small.tile([128, 1], BF16,
                                                         tag="mxb",
                                                         name="mxb")
                                        nc.vector.tensor_copy(mxb, mx[jn])
                                        nc.tensor.ldweights(mxb)
                                nc.vector.tensor_max(mx[0], mx[0], mx[1])
                                nc.vector.tensor_max(mx[2], mx[2], mx[3])
                                nc.vector.tensor_max(mx[0], mx[0], mx[2])
                                bias = small.tile([128, 1], F32, tag="bias",
                                                  name="bias")
                                nc.gpsimd.tensor_mul(bias, mx[0],
                                                     nrq_all[t][:, col])
                                attn = attn_pool.tile([128, S], BF16,
                                                      tag=f"attn{hh}",
                                                      name=f"attn{hh}")
                                for jn in range(NJ):
                                    jsl = slice(jn * 512, (jn + 1) * 512)
                                    nc.scalar.activation(
                                        out=attn[:, jsl], in_=dots[jn],
                                        func=AF.Exp, bias=bias,
                                        scale=rq_all[t][:, col])
                                for jq in range(SCH // 8):
                                    tp = tpps.tile([128, 1024], BF16,
                                                   tag="tp", name="tp")
                                    for j2 in range(8):
                                        jc = jq * 8 + j2
                                        nc.tensor.transpose(
                                            tp[:, j2 * 128:(j2 + 1) * 128],
                                            attn[:, jc * 128:(jc + 1) * 128],
                                            ident)
                                    # one strided copy per staging tile: the
                                    # 8 blocks land at jc*512+u*128 in attnT
                                    src = tp[:].rearrange("p (b c) -> p b c",
                                                          b=8)
                                    dst = bass.AP(
                                        tensor=attnT.tensor,
                                        offset=attnT.offset
                                        + jq * 8 * 512 + u * 128,
                                        ap=[attnT.ap[0], [512, 8], [1, 128]],
                                    )
                                    if jq % 2 == 0:
                                        nc.vector.tensor_copy(dst, src)
                                    else:
                                        nc.scalar.copy(dst, src)
                        for hh in range(2):
                            flush_av(pending[hh], 4)
                            av = avps.tile([VW, 512], F32, tag=f"av{hh}",
                                           name=f"av{hh}")
                            pending[hh] = (attnTs[hh], av, t, hh, sup)
                for part in range(5):
                    for hh in range(2):
                        flush_av(pending[hh], part)

        if DBG:
            nc.sync.dma_start(out=dbg_sm, in_=sums4[0])

        # ---- phase F2: normalize out^T by 1/sum ---------------------------
        with ExitStack() as phase_f2:
            bcps = phase_f2.enter_context(
                tc.tile_pool(name="bcps", bufs=4, space="PSUM"))
            rrow = phase_f2.enter_context(tc.tile_pool(name="rrow", bufs=1))
            # gather the 4 [1,S] sums rows over 64 partitions, one fast
            # reciprocal, scatter back to rows for the broadcast matmuls
            rsg = rrow.tile([64, 128], F32R, tag="rsg", name="rsg")
            for h in range(HPC):
                nc.sync.dma_start(
                    out=rsg[16 * h:16 * h + 16, :],
                    in_=sums4[h][0:1, :].rearrange("p (a b) -> p a b", a=16))
            nc.vector.reciprocal(rsg, rsg)
            rcp_rows = [rrow.tile([1, S], F32R, tag=f"rcp{h}",
                                  name=f"rcp{h}") for h in range(HPC)]
            for h in range(HPC):
                nc.scalar.dma_start(
                    out=rcp_rows[h][0:1, :].rearrange("p (a b) -> p a b",
                                                      a=16),
                    in_=rsg[16 * h:16 * h + 16, :])
            for t in range(NT):
                for nq in range(NJ):
                    nsl = slice(nq * 512, (nq + 1) * 512)
                    for hh in range(2):
                        bc = bcps.tile([64, 512], F32, tag="bc", name="bc")
                        nc.tensor.matmul(bc, e2t2r[0:1, 0:64],
                                         rcp_rows[2 * t + hh][:, nsl],
                                         start=True, stop=True)
                        osl = outT[t][hh * 64:(hh + 1) * 64, nsl]
                        nc.vector.tensor_mul(osl, osl, bc)

        # ---- phase G: output projection (bf16) ---------------------------
        with ExitStack() as phase_g:
            wops = phase_g.enter_context(
                tc.tile_pool(name="wops", bufs=4, space="PSUM"))
            gpool = phase_g.enter_context(tc.tile_pool(name="g", bufs=1))
            ostage = phase_g.enter_context(tc.tile_pool(name="ost", bufs=2))
            wo = [gpool.tile([128, DIM], BF16, tag=f"wo{k}", name=f"wo{k}")
                  for k in range(2)]
            for k in range(2):
                nc.sync.dma_start(out=wo[k], in_=wo_d[k * 128:(k + 1) * 128, :])
            for sc in range(SCH):
                ssl = slice(sc * 128, (sc + 1) * 128)
                ost = ostage.tile([128, DIM], F32, tag="ost", name="ost")
                for nn in range(2):
                    nsl = slice(nn * 512, (nn + 1) * 512)
                    ps = wops.tile([128, 512], F32, tag="wops", name="wops")
                    for kk in range(2):
                        nc.tensor.matmul(ps, outT[kk][:, ssl], wo[kk][:, nsl],
                                         start=(kk == 0), stop=(kk == 1))
                    if nn % 2 == 0:
                        nc.vector.tensor_copy(ost[:, nsl], ps)
                    else:
                        nc.scalar.copy(ost[:, nsl], ps)
                nc.sync.dma_start(out=out_d[ssl, :], in_=ost)

    _fix_multiwaits(nc)
    return nc


_NC = None


def _get_nc():
    global _NC
    if _NC is None:
        _NC = _build_program()
    return _NC


def kernel(x, ln_w, ln_b, Wq, Wkv, q_gamma, k_gamma, Wo):
    x = np.asarray(x, np.float32)
    ln_w = np.asarray(ln_w, np.float32)
    ln_b = np.asarray(ln_b, np.float32)
    Wq = np.asarray(Wq, np.float32)
    Wkv = np.asarray(Wkv, np.float32)
    q_gamma = np.asarray(q_gamma, np.float32)
    k_gamma = np.asarray(k_gamma, np.float32)
    Wo = np.asarray(Wo, np.float32)
    Wk_full = Wkv[:, :H * DH]
    Wv_full = Wkv[:, H * DH:]

    bf = ml_dtypes.bfloat16

    def hilo(a):
        hi = a.astype(bf)
        lo = (a - hi.astype(np.float32)).astype(bf)
        return hi, lo

    e2_host = np.zeros((128, 2), np.float32)
    e2_host[0:64, 0] = 1.0
    e2_host[64:128, 1] = 1.0
    e2t_host = np.ascontiguousarray(e2_host.T)

    def aug_weights(Wsl):
        # [1152, INC]: [ln_w*W; -colsum/1024; ln_b@W; zeros]
        Wt = ln_w[:, None] * Wsl
        out = np.zeros((9 * 128, INC), np.float32)
        out[:DIM] = Wt
        out[DIM] = -Wt.sum(axis=0) / DIM
        out[DIM + 1] = ln_b @ Wsl
        return out

    in_maps = []
    for c in range(NCORES):
        b = c // (NCORES // B)
        g0 = (c % (NCORES // B)) * HPC
        hsl = slice(g0 * DH, (g0 + HPC) * DH)
        xt_host = np.ascontiguousarray(x[b].T)
        wq_aug = aug_weights(Wq[:, hsl])
        wk_aug = aug_weights(Wk_full[:, hsl])
        wqh, wql = hilo(wq_aug[:DIM])
        wkh, wkl = hilo(wk_aug[:DIM])
        in_maps.append({
            "xT": xt_host,
            "Wqh": wqh, "Wql": wql,
            "Wkh": wkh, "Wkl": wkl,
            "Wqa": np.ascontiguousarray(wq_aug[DIM:DIM + 2]),
            "Wka": np.ascontiguousarray(wk_aug[DIM:DIM + 2]),
            "Wv": aug_weights(Wv_full[:, hsl]).astype(bf),
            "Wo": np.ascontiguousarray(Wo[hsl, :]).astype(bf),
            "gq": (8.0 * q_gamma[g0:g0 + HPC]).reshape(INC, 1).astype(np.float32),
            "gk": (8.0 * k_gamma[g0:g0 + HPC]).reshape(INC, 1).astype(np.float32),
            "E2": e2_host,
            "E2T": e2t_host,
            "E2TF": e2t_host,
        })

    res = run_bass_kernel_spmd(_get_nc(), in_maps, list(range(NCORES))).results
    gpb = NCORES // B
    out = np.zeros((B, S, DIM), np.float32)
    for b in range(B):
        acc = np.zeros((S, DIM), np.float32)
        for c in range(b * gpb, (b + 1) * gpb):
            acc += res[c]["out"]
        out[b] = acc
    return out


# revision 35
# speedup vs baseline: 1.1932x; 1.1932x over previous
"""Trainium2 Bass kernel for nn_Attention (dense transformer attention layer).

Sharding: 8 cores = 2 (batch) x 4 (head-group TP).  Core c handles batch
c//4 and heads [4*(c%4), 4*(c%4)+4).  Each core computes LayerNorm (in the
transposed domain, folded into augmented projection matmuls), q/k/v
projections, per-head RMS-norm'd attention, and a partial output
projection; the host sums the 4 partials per batch.

Precision: the dots have sigma~512 (scale=sqrt(dh) applied to BOTH q and k),
so the q/k path needs ~fp32 accuracy.  The host splits x and Wq/Wk into
bf16 hi/lo pairs; projections run as 3 bf16 passes (xh*wh + xl*wh + xh*wl,
3 cyc/col vs fp32's 4); QK^T runs as the exact bf16x2 pack scheme
(q=[hi;lo], k1=[hi;hi], k2=[lo;lo] -> all 4 cross terms in 2 bf16 matmuls).
Per-token/column scale factors (rk, rq, 1/sum) stay fp32 because scale
errors are amplified by |dots|~1700.  The v path, attention weights, and
Wo run in bf16.

Layout notes:
 - x is fed transposed (x^T [DIM, S]) so all matmuls contract over
   partitions without on-device transposition of x.
 - LayerNorm: token sums via bf16 ones-matmuls over xh+xl; sums of squares
   via a bf16 ones-matmul over square(xh); folded into the projections
   via two augmented contraction rows: z = [x^T; colsums; 1/r] with the
   aug matmul in f32r, W = [ln_w*W; -u/1024; ln_b@W].  The per-token 1/r
   cancels in q/k (RMSNorm scale invariance) and is applied explicitly to v.
 - v is projected in NATURAL orientation (stationary = bf16 x^T chunk,
   moving = Wv) so no PE transposes of v are needed; each v_nat tile also
   carries a ones column per head so the attention AV matmul produces the
   softmax denominators for free in psum row 64.
 - The bf16x2 packs are built with SBUF->SBUF DMAs (partition moves), no
   DRAM roundtrip.
 - RMSNorm of q is folded into the softmax exp (ACT scale/bias are
   per-partition APs); RMSNorm of k and gamma_k are applied via a single
   fused scalar_tensor_tensor pass.
"""
import numpy as np
import ml_dtypes
from contextlib import ExitStack

import concourse.bass as bass
import concourse.tile as tile
from concourse import mybir
from concourse.bass_utils import run_bass_kernel_spmd
from concourse.masks import make_identity

F32 = mybir.dt.float32
F32R = mybir.dt.float32r
BF16 = mybir.dt.bfloat16
AF = mybir.ActivationFunctionType
ALU = mybir.AluOpType
AX = mybir.AxisListType

B, S, DIM, H, DH = 2, 2048, 1024, 16, 64
NCORES = 8
HPC = 4                  # heads per core
INC = HPC * DH           # 256 inner dims per core
KCH = DIM // 128         # 8 contraction chunks of x
NT = 2                   # q/k^T tiles per core ([128, S] each, 2 heads per tile)
SCH = S // 128           # 16 token chunks
NJ = S // 512            # 4 moving chunks
VW = DH + 1              # v_nat column stride per head (64 v dims + ones col)

_TPB_ENGINES = None


def _fix_multiwaits(nc, max_waits=1):
    """walrus in this container encodes at most one semaphore wait per TPB
    instruction; split extras onto single-wait NoOps ahead of the
    instruction (same engine => program order preserves semantics)."""
    global _TPB_ENGINES
    if _TPB_ENGINES is None:
        _TPB_ENGINES = {
            mybir.EngineType.PE,
            mybir.EngineType.Activation,
            mybir.EngineType.DVE,
            mybir.EngineType.Pool,
            mybir.EngineType.SP,
        }
    nsplit = 0
    for f in nc.m.functions:
        for bb in f.blocks:
            new = []
            for inst in bb.instructions:
                si = inst.sync_info
                if (
                    inst.engine in _TPB_ENGINES
                    and si is not None
                    and si.on_wait
                    and len(si.on_wait) > max_waits
                ):
                    waits = list(si.on_wait)
                    extra, keep = waits[:-max_waits], waits[-max_waits:]
                    for w in extra:
                        nop = mybir.InstNoOp(
                            name=f"I-{nc.next_id()}",
                            ins=[],
                            outs=[],
                            engine=inst.engine,
                            sync_info=mybir.SyncInfo(on_wait=[w], on_update=[]),
                        )
                        try:
                            nc.register_instruction(nop, overwrite=True)
                        except Exception:
                            pass
                        new.append(nop)
                    try:
                        si.on_wait[:] = keep
                    except TypeError:
                        inst.sync_info = mybir.SyncInfo(
                            on_wait=keep, on_update=si.on_update
                        )
                    nsplit += 1
                new.append(inst)
            bb.instructions[:] = new
    return nsplit


def _build_program():
    nc = bass.Bass("TRN2", target_bir_lowering=False, debug=False,
                   num_devices=NCORES)
    din = lambda n, s, d: nc.dram_tensor(n, list(s), d, kind="ExternalInput").ap()
    xT_d = din("xT", (DIM, S), F32R)
    wqh_d = din("Wqh", (DIM, INC), BF16)
    wql_d = din("Wql", (DIM, INC), BF16)
    wkh_d = din("Wkh", (DIM, INC), BF16)
    wkl_d = din("Wkl", (DIM, INC), BF16)
    wqa_d = din("Wqa", (2, INC), F32R)
    wka_d = din("Wka", (2, INC), F32R)
    wv_d = din("Wv", (9 * 128, INC), BF16)
    wo_d = din("Wo", (INC, DIM), BF16)
    gq_d = din("gq", (INC, 1), F32)
    gk_d = din("gk", (INC, 1), F32)
    e2_d = din("E2", (128, 2), F32R)
    e2t_d = din("E2T", (2, 128), F32R)
    e2tf_d = din("E2TF", (2, 128), F32)
    out_d = nc.dram_tensor("out", [S, DIM], F32, kind="ExternalOutput").ap()
    import os
    DBG = bool(os.environ.get("KDBG"))
    if DBG:
        dbg_qh = nc.dram_tensor("dbg_qh", [128, S], F32,
                                kind="ExternalOutput").ap()
        dbg_kh = nc.dram_tensor("dbg_kh", [128, S], F32,
                                kind="ExternalOutput").ap()
        dbg_qp = nc.dram_tensor("dbg_qp", [128, S], BF16,
                                kind="ExternalOutput").ap()
        dbg_k1 = nc.dram_tensor("dbg_k1", [128, S], BF16,
                                kind="ExternalOutput").ap()
        dbg_vn = nc.dram_tensor("dbg_vn", [128, HPC * VW], BF16,
                                kind="ExternalOutput").ap()
        dbg_sm = nc.dram_tensor("dbg_sm", [1, S], F32,
                                kind="ExternalOutput").ap()
        dbg_ag = nc.dram_tensor("dbg_ag", [2, S], F32,
                                kind="ExternalOutput").ap()
        dbg_rs = nc.dram_tensor("dbg_rs", [1, S], F32,
                                kind="ExternalOutput").ap()
        dbg_xh = nc.dram_tensor("dbg_xh", [128, S], BF16,
                                kind="ExternalOutput").ap()
        dbg_wv = nc.dram_tensor("dbg_wv", [128, INC], BF16,
                                kind="ExternalOutput").ap()

    with tile.TileContext(nc) as tc, ExitStack() as ctx, \
            nc.allow_low_precision(reason="f32r/bf16 rounding intended"):
        # ---- long-lived pools (entered first: strict LIFO pool release)
        consts = ctx.enter_context(tc.tile_pool(name="consts", bufs=1))
        qkpool = ctx.enter_context(tc.tile_pool(name="qk", bufs=1))
        vpool = ctx.enter_context(tc.tile_pool(name="v", bufs=1))
        opool = ctx.enter_context(tc.tile_pool(name="o", bufs=1))

        ident = consts.tile([128, 128], BF16, tag="ident", name="ident")
        make_identity(nc, ident)
        one1 = consts.tile([1, 1], F32, tag="one1", name="one1")
        nc.vector.memset(one1, 1.0)
        eps1 = consts.tile([1, 1], F32, tag="eps1", name="eps1")
        nc.vector.memset(eps1, 1e-5)
        c15 = consts.tile([2, 1], F32, tag="c15", name="c15")
        nc.vector.memset(c15, 1.5)
        eps8 = consts.tile([2, 1], F32, tag="eps8", name="eps8")
        nc.vector.memset(eps8, 1e-8)
        ones_b = consts.tile([128, 1], BF16, tag="ones", name="ones")
        nc.vector.memset(ones_b, 1.0)
        e2 = consts.tile([128, 2], F32R, tag="e2", name="e2")
        nc.sync.dma_start(out=e2, in_=e2_d[:])
        e2f = consts.tile([128, 2], F32, tag="e2f", name="e2f")
        nc.gpsimd.dma_start(out=e2f, in_=e2)
        e2t2r = consts.tile([2, 128], F32R, tag="e2t2r", name="e2t2r")
        nc.sync.dma_start(out=e2t2r, in_=e2t_d[:])
        e2t2f = consts.tile([2, 128], F32, tag="e2t2f", name="e2t2f")
        nc.sync.dma_start(out=e2t2f, in_=e2tf_d[:])
        r_col = consts.tile([128, SCH], F32, tag="r_col", name="r_col")
        gq_t = [consts.tile([128, 1], F32, tag=f"gq{t}", name=f"gq{t}")
                for t in range(NT)]
        gk_t = [consts.tile([128, 1], F32, tag=f"gk{t}", name=f"gk{t}")
                for t in range(NT)]
        for t in range(NT):
            nc.sync.dma_start(out=gq_t[t], in_=gq_d[t * 128:(t + 1) * 128, :])
            nc.sync.dma_start(out=gk_t[t], in_=gk_d[t * 128:(t + 1) * 128, :])
        # rmsnorm factors for q (per-partition layout; col = 2*sc+hh)
        rq_all = [consts.tile([128, 2 * SCH], F32, tag=f"rq{t}", name=f"rq{t}")
                  for t in range(NT)]
        nrq_all = [consts.tile([128, 2 * SCH], F32, tag=f"nrq{t}", name=f"nrq{t}")
                   for t in range(NT)]

        qhat = [qkpool.tile([128, S], F32, tag=f"qh{t}", name=f"qh{t}")
                for t in range(NT)]
        khat = [qkpool.tile([128, S], F32, tag=f"kh{t}", name=f"kh{t}")
                for t in range(NT)]
        # v in natural orientation [tokens, 4 heads x (64 dims + ones col)]
        v_nat = [vpool.tile([128, HPC * VW], BF16, tag=f"vn{j}", name=f"vn{j}")
                 for j in range(SCH)]
        outT = [opool.tile([128, S], BF16, tag=f"oT{k}", name=f"oT{k}")
                for k in range(NT)]
        # softmax denominators, one row tile per head, filled by phase F
        sums4 = [opool.tile([1, S], F32R, tag=f"sums{h}", name=f"sums{h}")
                 for h in range(HPC)]

        with ExitStack() as phase_pre:
            augpool = phase_pre.enter_context(tc.tile_pool(name="aug", bufs=1))
            aug_f = augpool.tile([2, S], F32R, tag="aug_f", name="aug_f")
            r_sb = augpool.tile([1, S], F32, tag="r_sb", name="r_sb")
            wvpool = phase_pre.enter_context(tc.tile_pool(name="wv", bufs=1))
            wv = [wvpool.tile([128, INC], BF16, tag=f"wv{k}", name=f"wv{k}")
                  for k in range(9)]
            aug_b = wvpool.tile([2, S], BF16, tag="aug_b", name="aug_b")

            with ExitStack() as phase_bx:
                xpool = phase_bx.enter_context(tc.tile_pool(name="x", bufs=1))
                xh = [xpool.tile([128, S], BF16, tag=f"xh{k}", name=f"xh{k}")
                      for k in range(KCH)]
                xl = [xpool.tile([128, S], BF16, tag=f"xl{k}", name=f"xl{k}")
                      for k in range(KCH)]

                # ---- phase B: LayerNorm stats -----------------------------
                with ExitStack() as phase_b:
                    x2pool = phase_b.enter_context(
                        tc.tile_pool(name="x2", bufs=1))
                    browp = phase_b.enter_context(
                        tc.tile_pool(name="brow", bufs=1))
                    stps = phase_b.enter_context(
                        tc.tile_pool(name="stps", bufs=1, space="PSUM"))
                    sums_ps = stps.tile([1, S], F32, tag="sums", name="sums")
                    sumsq_ps = stps.tile([1, S], F32, tag="sumsq", name="sumsq")
                    xtp = phase_b.enter_context(
                        tc.tile_pool(name="xtp", bufs=2))
                    ones_f = consts.tile([128, 1], F32R, tag="ones_f",
                                         name="ones_f")
                    nc.vector.tensor_add(ones_f, e2[:, 0:1], e2[:, 1:2])
                    for k in range(KCH):
                        xt = xtp.tile([128, S], F32R, tag="xt", name="xt")
                        nc.sync.dma_start(out=xt,
                                          in_=xT_d[k * 128:(k + 1) * 128, :])
                        # on-device bf16 hi/lo split (large bf16 inputs get
                        # mangled by the XLA 16-bit tiled layout)
                        nc.scalar.copy(xh[k], xt)
                        nc.vector.tensor_sub(xl[k], xt, xh[k])
                        x2 = x2pool.tile([128, S], BF16, tag="x2", name="x2")
                        nc.scalar.square(x2, xh[k])
                        for n in range(NJ):
                            nsl = slice(n * 512, (n + 1) * 512)
                            nc.tensor.matmul(sums_ps[:, nsl], ones_f,
                                             xt[:, nsl],
                                             start=(k == 0),
                                             stop=(k == KCH - 1),
                                             skip_group_check=True)
                        for n in range(NJ):
                            nsl = slice(n * 512, (n + 1) * 512)
                            nc.tensor.matmul(sumsq_ps[:, nsl], ones_b,
                                             x2[:, nsl],
                                             start=(k == 0),
                                             stop=(k == KCH - 1),
                                             skip_group_check=True)
                    for k in range(9):
                        nc.sync.dma_start(out=wv[k],
                                          in_=wv_d[k * 128:(k + 1) * 128, :])
                    # sigma = sqrt((sumsq - sums^2/1024)/1024 + 1e-5).
                    # [1,S] rows run on ONE lane, so minimize row ops and
                    # fold scales into ACT square/sqrt.
                    nc.scalar.activation(out=r_sb, in_=sums_ps,
                                         func=AF.Square, bias=0.0,
                                         scale=1.0 / 32.0)
                    nc.vector.tensor_sub(r_sb, sumsq_ps, r_sb)
                    sig_row = browp.tile([1, S], F32, tag="sig", name="sig")
                    nc.scalar.activation(out=sig_row, in_=r_sb, func=AF.Sqrt,
                                         bias=eps1[0:1, 0:1],
                                         scale=1.0 / DIM)
                    nc.vector.tensor_copy(aug_f[0:1, :], sums_ps)
                    # invr = sigma  (aug row 1; gpsimd dma does the cast)
                    nc.gpsimd.dma_start(out=aug_f[1:2, :], in_=sig_row)
                nc.gpsimd.dma_start(out=aug_b, in_=aug_f)
                if DBG:
                    nc.gpsimd.dma_start(out=dbg_ag, in_=aug_f)
                    nc.sync.dma_start(out=dbg_rs, in_=r_sb)
                    nc.sync.dma_start(out=dbg_xh, in_=xh[0])
                    nc.sync.dma_start(out=dbg_wv, in_=wv[0])

                # r as a column [128 tokens, SCH]
                with ExitStack() as phase_rc:
                    rtp = phase_rc.enter_context(
                        tc.tile_pool(name="rtp", bufs=2, space="PSUM"))
                    sig_col = consts.tile([128, SCH], F32, tag="sig_col",
                                          name="sig_col")
                    for j in range(SCH):
                        rp = rtp.tile([128, 1], F32, tag="rp", name="rp")
                        nc.tensor.transpose(rp,
                                            sig_row[0:1,
                                                    j * 128:(j + 1) * 128],
                                            one1[0:1, 0:1])
                        nc.vector.tensor_copy(sig_col[:, j:j + 1], rp)
                    nc.vector.reciprocal(r_col, sig_col)

                # ---- phase C: q/k projections (3-pass bf16x2) -------------
                with ExitStack() as phase_c:
                    wqkpool = phase_c.enter_context(
                        tc.tile_pool(name="wqk", bufs=1))
                    prps = phase_c.enter_context(
                        tc.tile_pool(name="prps", bufs=4, space="PSUM"))
                    for wh_d, wl_d, wa_d, dst in (
                            (wqh_d, wql_d, wqa_d, qhat),
                            (wkh_d, wkl_d, wka_d, khat)):
                        wh = [wqkpool.tile([128, INC], BF16, tag=f"wh{k}",
                                           name=f"wh{k}") for k in range(KCH)]
                        wl = [wqkpool.tile([128, INC], BF16, tag=f"wl{k}",
                                           name=f"wl{k}") for k in range(KCH)]
                        wa = wqkpool.tile([2, INC], F32R, tag="wa", name="wa")
                        for k in range(KCH):
                            nc.sync.dma_start(
                                out=wh[k], in_=wh_d[k * 128:(k + 1) * 128, :])
                            nc.sync.dma_start(
                                out=wl[k], in_=wl_d[k * 128:(k + 1) * 128, :])
                        nc.sync.dma_start(out=wa, in_=wa_d[:])
                        for m in range(NT):
                            msl = slice(m * 128, (m + 1) * 128)
                            for n in range(NJ):
                                nsl = slice(n * 512, (n + 1) * 512)
                                ps = prps.tile([128, 512], F32, tag="proj",
                                               name="proj")
                                for k in range(KCH):
                                    nc.tensor.matmul(ps, wh[k][:, msl],
                                                     xh[k][:, nsl],
                                                     start=(k == 0),
                                                     stop=False)
                                    nc.tensor.matmul(ps, wh[k][:, msl],
                                                     xl[k][:, nsl],
                                                     start=False, stop=False)
                                    nc.tensor.matmul(ps, wl[k][:, msl],
                                                     xh[k][:, nsl],
                                                     start=False, stop=False)
                                nc.tensor.matmul(ps, wa[0:2, msl],
                                                 aug_f[:, nsl],
                                                 start=False, stop=True)
                                if n % 2 == 0:
                                    nc.vector.tensor_copy(dst[m][:, nsl], ps)
                                else:
                                    nc.scalar.copy(dst[m][:, nsl], ps)

                # ---- phase E: v projection (bf16, natural orientation) ----
                with ExitStack() as phase_e:
                    vprps = phase_e.enter_context(
                        tc.tile_pool(name="vprps", bufs=2, space="PSUM"))
                    for jc in range(SCH):
                        jsl = slice(jc * 128, (jc + 1) * 128)
                        vp = vprps.tile([128, INC], F32, tag="vp", name="vp")
                        for k in range(KCH):
                            nc.tensor.matmul(vp, xh[k][:, jsl], wv[k],
                                             start=(k == 0), stop=False)
                        nc.tensor.matmul(vp, aug_b[:, jsl], wv[8][0:2, :],
                                         start=False, stop=True)
                        # fold 1/sigma into the psum->sbuf copy; strided dst
                        # leaves a ones column per head for the denominator
                        dst = bass.AP(
                            tensor=v_nat[jc].tensor,
                            offset=v_nat[jc].offset,
                            ap=[v_nat[jc].ap[0], [VW, HPC], [1, DH]],
                        )
                        src = vp[:].rearrange("p (h d) -> p h d", h=HPC)
                        nc.vector.tensor_scalar_mul(dst, src,
                                                    r_col[:, jc:jc + 1])
                        onescol = bass.AP(
                            tensor=v_nat[jc].tensor,
                            offset=v_nat[jc].offset + DH,
                            ap=[v_nat[jc].ap[0], [VW, HPC], [1, 1]],
                        )
                        nc.gpsimd.memset(onescol, 1.0)

            # ---- phase D: rmsnorm factors -----------------------------
            with ExitStack() as phase_d:
                sq2pool = phase_d.enter_context(
                    tc.tile_pool(name="sq2", bufs=2))
                dstage = phase_d.enter_context(
                    tc.tile_pool(name="dstage", bufs=1))
                ssqps = phase_d.enter_context(
                    tc.tile_pool(name="ssqps", bufs=2, space="PSUM"))
                sskps = phase_d.enter_context(
                    tc.tile_pool(name="sskps", bufs=1, space="PSUM"))
                kmps = phase_d.enter_context(
                    tc.tile_pool(name="kmps", bufs=2, space="PSUM"))
                for t in range(NT):
                    q2 = sq2pool.tile([128, S], F32R, tag="q2", name="q2")
                    nc.gpsimd.tensor_mul(q2, qhat[t], qhat[t])
                    for sc in range(SCH):
                        psq = ssqps.tile([128, 2], F32, tag="ssq", name="ssq")
                        nc.tensor.matmul(psq, q2[:, sc * 128:(sc + 1) * 128],
                                         e2, start=True, stop=True)
                        nc.vector.tensor_copy(
                            rq_all[t][:, 2 * sc:2 * sc + 2], psq)
                    # rq = rsqrt(ss/64 + 1e-8), Newton-refined
                    tq = dstage.tile([128, 2 * SCH], F32, tag="tq", name="tq")
                    nc.vector.tensor_scalar(tq, rq_all[t], 1.0 / DH, 1e-8,
                                            op0=ALU.mult, op1=ALU.add)
                    nc.scalar.activation(out=rq_all[t], in_=tq, func=AF.Sqrt,
                                         bias=0.0, scale=1.0)
                    nc.vector.reciprocal(rq_all[t], rq_all[t])
                    zz = dstage.tile([128, 2 * SCH], F32, tag="zz", name="zz")
                    nc.vector.tensor_mul(zz, rq_all[t], rq_all[t])
                    nc.vector.tensor_mul(zz, zz, tq)
                    nc.vector.tensor_scalar(zz, zz, -0.5, 1.5,
                                            op0=ALU.mult, op1=ALU.add)
                    nc.vector.tensor_mul(rq_all[t], rq_all[t], zz)
                    nc.vector.tensor_scalar_mul(nrq_all[t], rq_all[t], -1.0)
                    # gamma*8 on q
                    nc.vector.tensor_scalar_mul(qhat[t], qhat[t], gq_t[t])

                    k2t = sq2pool.tile([128, S], F32, tag="q2", name="k2t")
                    nc.gpsimd.tensor_mul(k2t, khat[t], khat[t])
                    psk = sskps.tile([2, S], F32, tag="ssk", name="ssk")
                    for n in range(NJ):
                        nsl = slice(n * 512, (n + 1) * 512)
                        nc.tensor.matmul(psk[:, nsl], e2f, k2t[:, nsl],
                                         start=True, stop=True,
                                         skip_group_check=True)
                    # rk = rsqrt(psk/64 + 1e-8).  [2,S] rows run on <=2
                    # DVE lanes, so gather the rows across 32 partitions via
                    # SBUF<->SBUF DMA, do sqrt+recip+Newton there (fast),
                    # and scatter back to rows for the broadcast matmul.
                    tks = dstage.tile([2, S], F32, tag="tks", name="tks")
                    nc.vector.tensor_scalar(tks, psk, 1.0 / DH, 1e-8,
                                            op0=ALU.mult, op1=ALU.add)
                    tg = dstage.tile([32, 128], F32, tag="tg", name="tg")
                    for rr in range(2):
                        nc.sync.dma_start(
                            out=tg[16 * rr:16 * rr + 16, :],
                            in_=tks[rr:rr + 1, :].rearrange(
                                "p (a b) -> p a b", a=16))
                    rg = dstage.tile([32, 128], F32, tag="rg", name="rg")
                    nc.scalar.activation(out=rg, in_=tg, func=AF.Sqrt,
                                         bias=0.0, scale=1.0)
                    nc.vector.reciprocal(rg, rg)
                    zg = dstage.tile([32, 128], F32, tag="zg", name="zg")
                    nc.vector.tensor_mul(zg, rg, rg)
                    nc.vector.tensor_mul(zg, zg, tg)
                    nc.vector.tensor_scalar(zg, zg, -0.5, 1.5,
                                            op0=ALU.mult, op1=ALU.add)
                    nc.vector.tensor_mul(rg, rg, zg)
                    rk2 = dstage.tile([2, S], F32, tag="rk2", name="rk2")
                    for rr in range(2):
                        nc.scalar.dma_start(
                            out=rk2[rr:rr + 1, :].rearrange(
                                "p (a b) -> p a b", a=16),
                            in_=rg[16 * rr:16 * rr + 16, :])
                    # k *= gamma*8 (per-partition) and rk (fp32 broadcast over
                    # partitions via K=2 matmul), fused in one DVE pass
                    for n in range(NJ):
                        nsl = slice(n * 512, (n + 1) * 512)
                        km = kmps.tile([128, 512], F32, tag="km", name="km")
                        nc.tensor.matmul(km, e2t2f, rk2[:, nsl],
                                         start=True, stop=True)
                        nc.vector.scalar_tensor_tensor(
                            khat[t][:, nsl], khat[t][:, nsl], gk_t[t], km,
                            op0=ALU.mult, op1=ALU.mult)

        # ---- bf16x2 packs + attention --------------------------------------
        with ExitStack() as phase_post:
            packp = phase_post.enter_context(tc.tile_pool(name="packs",
                                                          bufs=1))
            q_pack = [packp.tile([128, S], BF16, tag=f"qp{h}", name=f"qp{h}")
                      for h in range(HPC)]
            k1_pack = [packp.tile([128, S], BF16, tag=f"k1p{h}",
                                  name=f"k1p{h}") for h in range(HPC)]
            k2_pack = [packp.tile([128, S], BF16, tag=f"k2p{h}",
                                  name=f"k2p{h}") for h in range(HPC)]
            with ExitStack() as phase_pk:
                hilop = phase_pk.enter_context(tc.tile_pool(name="hilo",
                                                            bufs=1))
                qhi = [hilop.tile([128, S], BF16, tag=f"qhi{t}",
                                  name=f"qhi{t}") for t in range(NT)]
                qlo = [hilop.tile([128, S], BF16, tag=f"qlo{t}",
                                  name=f"qlo{t}") for t in range(NT)]
                khi = [hilop.tile([128, S], BF16, tag=f"khi{t}",
                                  name=f"khi{t}") for t in range(NT)]
                klo = [hilop.tile([128, S], BF16, tag=f"klo{t}",
                                  name=f"klo{t}") for t in range(NT)]
                for t in range(NT):
                    nc.scalar.copy(qhi[t], qhat[t])
                    nc.vector.tensor_sub(qlo[t], qhat[t], qhi[t])
                    nc.scalar.copy(khi[t], khat[t])
                    nc.vector.tensor_sub(klo[t], khat[t], khi[t])
                qs = [nc.sync, nc.scalar, nc.gpsimd]
                qi = 0
                for t in range(NT):
                    for hh in range(2):
                        h4 = 2 * t + hh
                        rows = slice(hh * 64, hh * 64 + 64)
                        moves = [
                            (q_pack[h4][0:64, :], qhi[t][rows, :]),
                            (q_pack[h4][64:128, :], qlo[t][rows, :]),
                            (k1_pack[h4][0:64, :], khi[t][rows, :]),
                            (k1_pack[h4][64:128, :], khi[t][rows, :]),
                            (k2_pack[h4][0:64, :], klo[t][rows, :]),
                            (k2_pack[h4][64:128, :], klo[t][rows, :]),
                        ]
                        for o, i in moves:
                            qs[qi % 3].dma_start(out=o, in_=i)
                            qi += 1

            if DBG:
                nc.sync.dma_start(out=dbg_qh, in_=qhat[0])
                nc.sync.dma_start(out=dbg_kh, in_=khat[0])
                nc.sync.dma_start(out=dbg_qp, in_=q_pack[0])
                nc.sync.dma_start(out=dbg_k1, in_=k1_pack[0])
                nc.sync.dma_start(out=dbg_vn, in_=v_nat[0])

            # ---- phase F: attention --------------------------------------
            with ExitStack() as phase_f:
                dots_pool = phase_f.enter_context(
                    tc.tile_pool(name="dots", bufs=6, space="PSUM"))
                tpps = phase_f.enter_context(
                    tc.tile_pool(name="tpps", bufs=1, space="PSUM"))
                avps = phase_f.enter_context(
                    tc.tile_pool(name="avps", bufs=1, space="PSUM"))
                attn_pool = phase_f.enter_context(tc.tile_pool(name="attn",
                                                               bufs=4))
                attnT_pool = phase_f.enter_context(tc.tile_pool(name="attnT",
                                                                bufs=2))
                small = phase_f.enter_context(tc.tile_pool(name="small",
                                                           bufs=8))

                def flush_av(pend, part):
                    # part<4: emit av matmul chunk (4 of 16 jc) for the
                    # pending sup; part==4 -> psum->sbuf copies
                    if pend is None:
                        return
                    p_attnT, p_av, p_t, p_hh, p_sup = pend
                    p_h4 = 2 * p_t + p_hh
                    if part < 4:
                        for jc in range(4 * part, 4 * part + 4):
                            nc.tensor.matmul(
                                p_av,
                                v_nat[jc][:, p_h4 * VW:(p_h4 + 1) * VW],
                                p_attnT[:, jc * 512:(jc + 1) * 512],
                                start=(jc == 0), stop=(jc == SCH - 1))
                    else:
                        poff = p_hh * 64
                        pssl = slice(p_sup * 512, (p_sup + 1) * 512)
                        if p_sup % 2 == 0:
                            nc.vector.tensor_copy(
                                outT[p_t][poff:poff + 64, pssl],
                                p_av[0:DH, :])
                            nc.scalar.copy(sums4[p_h4][:, pssl],
                                           p_av[DH:DH + 1, :])
                        else:
                            nc.scalar.copy(
                                outT[p_t][poff:poff + 64, pssl],
                                p_av[0:DH, :])
                            nc.vector.tensor_copy(sums4[p_h4][:, pssl],
                                                  p_av[DH:DH + 1, :])

                pending = None
                for t in range(NT):
                    for hh in range(2):
                        h4 = 2 * t + hh
                        for sup in range(SCH // 4):
                            attnT = attnT_pool.tile([128, 4 * S], BF16,
                                                    tag="attnT", name="attnT")
                            for u in range(4):
                                ic = sup * 4 + u
                                isl = slice(ic * 128, (ic + 1) * 128)
                                col = slice(2 * ic + hh, 2 * ic + hh + 1)
                                dots = [dots_pool.tile([128, 512], F32,
                                                       tag="dots",
                                                       name="dots")
                                        for _ in range(NJ)]
                                for jn in range(NJ):
                                    jsl = slice(jn * 512, (jn + 1) * 512)
                                    nc.tensor.matmul(
                                        dots[jn], q_pack[h4][:, isl],
                                        k1_pack[h4][:, jsl],
                                        start=True, stop=False,
                                        skip_group_check=True)
                                    nc.tensor.matmul(
                                        dots[jn], q_pack[h4][:, isl],
                                        k2_pack[h4][:, jsl],
                                        start=False, stop=True,
                                        skip_group_check=True)
                                # PE stall-filler: one av chunk of the
                                # previous sup between dots and transposes
                                flush_av(pending, u)
                                mx = [small.tile([128, 1], F32, tag=f"mx{j}",
                                                 name=f"mx{j}")
                                      for j in range(NJ)]
                                for jn in range(NJ):
                                    nc.vector.tensor_reduce(out=mx[jn],
                                                            in_=dots[jn],
                                                            axis=AX.X,
                                                            op=ALU.max)
                                    if jn in (1, 3):
                                        # HAM keep-warm: a 1-column
                                        # LDWEIGHTS gated on a reduce pulses
                                        # the PE mid-gap so the idle window
                                        # never completes (K=8/8)
                                        mxb = small.tile([128, 1], BF16,
                                                         tag="mxb",
                                                         name="mxb")
                                        nc.vector.tensor_copy(mxb, mx[jn])
                                        nc.tensor.ldweights(mxb)
                                nc.vector.tensor_max(mx[0], mx[0], mx[1])
                                nc.vector.tensor_max(mx[2], mx[2], mx[3])
                                nc.vector.tensor_max(mx[0], mx[0], mx[2])
                                bias = small.tile([128, 1], F32, tag="bias",
                                                  name="bias")
                                nc.gpsimd.tensor_mul(bias, mx[0],
                                                     nrq_all[t][:, col])
                                attn = attn_pool.tile([128, S], BF16,
                                                      tag="attn", name="attn")
                                for jn in range(NJ):
                                    jsl = slice(jn * 512, (jn + 1) * 512)
                                    nc.scalar.activation(
                                        out=attn[:, jsl], in_=dots[jn],
                                        func=AF.Exp, bias=bias,
                                        scale=rq_all[t][:, col])
                                for jq in range(SCH // 8):
                                    tp = tpps.tile([128, 1024], BF16,
                                                   tag="tp", name="tp")
                                    for j2 in range(8):
                                        jc = jq * 8 + j2
                                        nc.tensor.transpose(
                                            tp[:, j2 * 128:(j2 + 1) * 128],
                                            attn[:, jc * 128:(jc + 1) * 128],
                                            ident)
                                    # one strided copy per staging tile: the
                                    # 8 blocks land at jc*512+u*128 in attnT
                                    src = tp[:].rearrange("p (b c) -> p b c",
                                                          b=8)
                                    dst = bass.AP(
                                        tensor=attnT.tensor,
                                        offset=attnT.offset
                                        + jq * 8 * 512 + u * 128,
                                        ap=[attnT.ap[0], [512, 8], [1, 128]],
                                    )
                                    if jq % 2 == 0:
                                        nc.vector.tensor_copy(dst, src)
                                    else:
                                        nc.scalar.copy(dst, src)
                            flush_av(pending, 4)
                            av = avps.tile([VW, 512], F32, tag="av",
                                           name="av")
                            pending = (attnT, av, t, hh, sup)
                for part in range(5):
                    flush_av(pending, part)

        if DBG:
            nc.sync.dma_start(out=dbg_sm, in_=sums4[0])

        # ---- phase F2: normalize out^T by 1/sum ---------------------------
        with ExitStack() as phase_f2:
            bcps = phase_f2.enter_context(
                tc.tile_pool(name="bcps", bufs=4, space="PSUM"))
            rrow = phase_f2.enter_context(tc.tile_pool(name="rrow", bufs=1))
            # gather the 4 [1,S] sums rows over 64 partitions, one fast
            # reciprocal, scatter back to rows for the broadcast matmuls
            rsg = rrow.tile([64, 128], F32R, tag="rsg", name="rsg")
            for h in range(HPC):
                nc.sync.dma_start(
                    out=rsg[16 * h:16 * h + 16, :],
                    in_=sums4[h][0:1, :].rearrange("p (a b) -> p a b", a=16))
            nc.vector.reciprocal(rsg, rsg)
            rcp_rows = [rrow.tile([1, S], F32R, tag=f"rcp{h}",
                                  name=f"rcp{h}") for h in range(HPC)]
            for h in range(HPC):
                nc.scalar.dma_start(
                    out=rcp_rows[h][0:1, :].rearrange("p (a b) -> p a b",
                                                      a=16),
                    in_=rsg[16 * h:16 * h + 16, :])
            for t in range(NT):
                for nq in range(NJ):
                    nsl = slice(nq * 512, (nq + 1) * 512)
                    for hh in range(2):
                        bc = bcps.tile([64, 512], F32, tag="bc", name="bc")
                        nc.tensor.matmul(bc, e2t2r[0:1, 0:64],
                                         rcp_rows[2 * t + hh][:, nsl],
                                         start=True, stop=True)
                        osl = outT[t][hh * 64:(hh + 1) * 64, nsl]
                        nc.vector.tensor_mul(osl, osl, bc)

        # ---- phase G: output projection (bf16) ---------------------------
        with ExitStack() as phase_g:
            wops = phase_g.enter_context(
                tc.tile_pool(name="wops", bufs=4, space="PSUM"))
            gpool = phase_g.enter_context(tc.tile_pool(name="g", bufs=1))
            ostage = phase_g.enter_context(tc.tile_pool(name="ost", bufs=2))
            wo = [gpool.tile([128, DIM], BF16, tag=f"wo{k}", name=f"wo{k}")
                  for k in range(2)]
            for k in range(2):
                nc.sync.dma_start(out=wo[k], in_=wo_d[k * 128:(k + 1) * 128, :])
            for sc in range(SCH):
                ssl = slice(sc * 128, (sc + 1) * 128)
                ost = ostage.tile([128, DIM], F32, tag="ost", name="ost")
                for nn in range(2):
                    nsl = slice(nn * 512, (nn + 1) * 512)
                    ps = wops.tile([128, 512], F32, tag="wops", name="wops")
                    for kk in range(2):
                        nc.tensor.matmul(ps, outT[kk][:, ssl], wo[kk][:, nsl],
                                         start=(kk == 0), stop=(kk == 1))
                    if nn % 2 == 0:
                        nc.vector.tensor_copy(ost[:, nsl], ps)
                    else:
                        nc.scalar.copy(ost[:, nsl], ps)
                nc.sync.dma_start(out=out_d[ssl, :], in_=ost)

    _fix_multiwaits(nc)
    return nc


_NC = None


def _get_nc():
    global _NC
    if _NC is None:
        _NC = _build_program()
    return _NC


def kernel(x, ln_w, ln_b, Wq, Wkv, q_gamma, k_gamma, Wo):
    x = np.asarray(x, np.float32)
    ln_w = np.asarray(ln_w, np.float32)
    ln_b = np.asarray(ln_b, np.float32)
    Wq = np.asarray(Wq, np.float32)
    Wkv = np.asarray(Wkv, np.float32)
    q_gamma = np.asarray(q_gamma, np.float32)
    k_gamma = np.asarray(k_gamma, np.float32)
    Wo = np.asarray(Wo, np.float32)
    Wk_full = Wkv[:, :H * DH]
    Wv_full = Wkv[:, H * DH:]

    bf = ml_dtypes.bfloat16

    def hilo(a):
        hi = a.astype(bf)
        lo = (a - hi.astype(np.float32)).astype(bf)
        return hi, lo

    e2_host = np.zeros((128, 2), np.float32)
    e2_host[0:64, 0] = 1.0
    e2_host[64:128, 1] = 1.0
    e2t_host = np.ascontiguousarray(e2_host.T)

    def aug_weights(Wsl):
        # [1152, INC]: [ln_w*W; -colsum/1024; ln_b@W; zeros]
        Wt = ln_w[:, None] * Wsl
        out = np.zeros((9 * 128, INC), np.float32)
        out[:DIM] = Wt
        out[DIM] = -Wt.sum(axis=0) / DIM
        out[DIM + 1] = ln_b @ Wsl
        return out

    in_maps = []
    for c in range(NCORES):
        b = c // (NCORES // B)
        g0 = (c % (NCORES // B)) * HPC
        hsl = slice(g0 * DH, (g0 + HPC) * DH)
        xt_host = np.ascontiguousarray(x[b].T)
        wq_aug = aug_weights(Wq[:, hsl])
        wk_aug = aug_weights(Wk_full[:, hsl])
        wqh, wql = hilo(wq_aug[:DIM])
        wkh, wkl = hilo(wk_aug[:DIM])
        in_maps.append({
            "xT": xt_host,
            "Wqh": wqh, "Wql": wql,
            "Wkh": wkh, "Wkl": wkl,
            "Wqa": np.ascontiguousarray(wq_aug[DIM:DIM + 2]),
            "Wka": np.ascontiguousarray(wk_aug[DIM:DIM + 2]),
            "Wv": aug_weights(Wv_full[:, hsl]).astype(bf),
            "Wo": np.ascontiguousarray(Wo[hsl, :]).astype(bf),
            "gq": (8.0 * q_gamma[g0:g0 + HPC]).reshape(INC, 1).astype(np.float32),
            "gk": (8.0 * k_gamma[g0:g0 + HPC]).reshape(INC, 1).astype(np.float32),
            "E2": e2_host,
            "E2T": e2t_host,
            "E2TF": e2t_host,
        })

    res = run_bass_kernel_spmd(_get_nc(), in_maps, list(range(NCORES))).results
    gpb = NCORES // B
    out = np.zeros((B, S, DIM), np.float32)
    for b in range(B):
        acc = np.zeros((S, DIM), np.float32)
        for c in range(b * gpb, (b + 1) * gpb):
            acc += res[c]["out"]
        out[b] = acc
    return out
